# revision 21
# baseline (speedup 1.0000x reference)
"""Trainium2 Bass kernel for multi-head dot-product attention (R=16 heads,
B=2, L=2048, D=1024) returning (att_v, att_groups), sharded over 8 NeuronCores.

Sharding: core c handles batch b = c//4 and head group g = c%4 (4 heads).
Each core computes its heads' projections (column-sliced Wq/Wk/Wv), full
attention for those heads, the attention-map output slice, and a partial
att_v (row-sliced Wc); the host sums the 4 partials per batch.

On-device dataflow (per core), matmuls in float32r (reduced-precision fp32,
full PE rate at N=512):
  A:   Q_projT/K_projT/V_projT [256,2048] via PE, streaming qT/kT/vT k-chunks
       from DRAM; V_projT is PE-transposed into natural V [lk, 65]-per-head
       tiles (65th column = ones for the row-sum-via-matmul trick).
  B_T: S^T chunks [128lk, 2048lq] -> exp (ScalarE) -> C: PE accumulates
       [V|1]^T @ expS^T = [O'^T; rowsums], drained blockwise to SBUF.
  B:   natural S chunks [128lq, 2048lk] -> exp with accumulated row sums
       (ScalarE accum_out) -> per-partition normalize (VectorE) -> DMA out.
  D:   att_v partial = concat_h(O'_norm) @ Wc_slice from O'^T slices.

Softmax skips the max-subtraction: scores are ~N(0, 2.6^2) (max |s| ~ 12),
far inside fp32 exp range, and exp(s)/sum(exp(s)) is algebraically identical
to the max-shifted form.
"""

import numpy as np

import concourse.bass as bass
import concourse.mybir as mybir
import concourse.tile as tile
from concourse import bacc, bass_utils
from concourse.masks import make_identity

B, L, D, R = 2, 2048, 1024, 16
HPC = 4          # heads per core
DR = D // R      # 64
WC = HPC * DR    # 256 = projected width per core
NCORES = 8
CH = L // 128    # 16 chunks of 128 along lq/lk
P = 128
NKC = D // P     # 8 contraction chunks for projections
F32 = mybir.dt.float32
F32R = mybir.dt.float32r
BF16 = mybir.dt.bfloat16
AF = mybir.ActivationFunctionType


def _emit(nc, tc, io):
    wp = tc.alloc_tile_pool(name="wp", bufs=1)
    xp = tc.alloc_tile_pool(name="xp", bufs=2)
    pp = tc.alloc_tile_pool(name="pp", bufs=1)
    ep = tc.alloc_tile_pool(name="ep", bufs=2)
    ap_ = tc.alloc_tile_pool(name="ap", bufs=3)
    sp = tc.alloc_tile_pool(name="sp", bufs=2)
    psp = tc.alloc_tile_pool(name="psp", bufs=4, space="PSUM")

    def ps_tile(name):
        return psp.tile([P, 1024], F32, name=name, tag="s")

    # ---- weights to SBUF ----
    wq_s = wp.tile([P, NKC, WC], F32R)
    wk_s = wp.tile([P, NKC, WC], F32R)
    wv_s = wp.tile([P, NKC, WC], F32R)
    wc_s = wp.tile([DR, HPC, D], F32R)
    nc.sync.dma_start(wq_s[:], io["wq"].rearrange("(k p) m -> p k m", p=P))
    nc.sync.dma_start(wk_s[:], io["wk"].rearrange("(k p) m -> p k m", p=P))
    nc.sync.dma_start(wv_s[:], io["wv"].rearrange("(k p) m -> p k m", p=P))
    nc.sync.dma_start(wc_s[:], io["wc"].rearrange("(j p) m -> p j m", p=DR))
    ident = wp.tile([P, P], F32)
    make_identity(nc, ident[:])
    ones32 = wp.tile([P, CH * HPC], F32)
    nc.vector.memset(ones32[:], 1.0)
    ones_col = wp.tile([1, DR], F32R)
    nc.vector.tensor_copy(ones_col[:], ones32[0:1, 0:DR])

    # ---- stage A: projections (transposed layout) ----
    # qp/kp: [128, 2, 2048]; (p, m, l) = Proj_T[m*128 + p, l]; head j lives at
    # partitions 64*(j%2)..+64 of m-tile j//2.
    qp = pp.tile([P, 2, L], F32R)
    kp = pp.tile([P, 2, L], F32R)
    # vp: natural-layout V per (lk-chunk, head), augmented with ones column:
    # vp[p, t, j, 0:64] = V_proj[t*128+p, 64j:64j+64], vp[p, t, j, 64] = 1.0
    vp = pp.tile([P, CH, HPC, DR + 1], F32R)
    nc.vector.tensor_copy(
        vp[:, :, :, DR : DR + 1],
        ones32[:].rearrange("p (t j o) -> p t j o", t=CH, j=HPC),
    )

    for name, wsrc, xdram, dst in (
        ("q", wq_s, io["qT"], qp),
        ("k", wk_s, io["kT"], kp),
        ("v", wv_s, io["vT"], None),
    ):
        # 4 PSUM groups (m, nh), all alive across the k accumulation
        grp = {
            (m, nh): ps_tile(f"ps_{name}_{m}_{nh}")
            for m in range(2)
            for nh in range(2)
        }
        for ki in range(NKC):
            xt = xp.tile([P, L], F32R, name=f"xt_{name}_{ki}", tag="xt")
            nc.sync.dma_start(xt[:, 0:1024], xdram[ki * P : (ki + 1) * P, 0:1024])
            nc.sync.dma_start(xt[:, 1024:2048], xdram[ki * P : (ki + 1) * P, 1024:2048])
            for m in range(2):
                for nh in range(2):
                    for n in range(2):
                        nc.tensor.matmul(
                            grp[(m, nh)][:, n * 512 : (n + 1) * 512],
                            wsrc[:, ki, m * P : (m + 1) * P],
                            xt[:, nh * 1024 + n * 512 : nh * 1024 + (n + 1) * 512],
                            start=(ki == 0),
                            stop=(ki == NKC - 1),
                        )
        if dst is not None:
            for m in range(2):
                for nh in range(2):
                    nc.vector.tensor_copy(
                        dst[:, m, nh * 1024 : (nh + 1) * 1024], grp[(m, nh)][:]
                    )
        else:
            # V: drain to transposed staging (f32), PE-transpose into vp
            vpT = []
            for m in range(2):
                vt = xp.tile([P, L], F32, name=f"vpT_{m}", tag="xt")
                vpT.append(vt)
                for nh in range(2):
                    nc.vector.tensor_copy(
                        vt[:, nh * 1024 : (nh + 1) * 1024], grp[(m, nh)][:]
                    )
            for j in range(HPC):
                src = vpT[j // 2]
                pb = 64 * (j % 2)
                tps = ps_tile(f"tps_{j}")
                for t in range(CH):
                    # out = in.T via regular matmul against an identity block
                    # (rhs base_partition must match the input slice's base)
                    nc.tensor.matmul(
                        tps[:, t * DR : (t + 1) * DR],
                        src[pb : pb + DR, t * P : (t + 1) * P],
                        ident[pb : pb + DR, pb : pb + DR],
                        start=True,
                        stop=True,
                    )
                nc.vector.tensor_copy(
                    vp[:, :, j, 0:DR],
                    tps[:].rearrange("p (t d) -> p t d", d=DR),
                )

    # ---- per-head attention ----
    # The transposed (B_T -> exp -> C) chain and the natural (B -> exp ->
    # normalize -> DMA) chain are dependency-independent; interleave their
    # emission unit by unit so the PE always has runnable matmuls while the
    # other chain waits on ScalarE. The per-head finalize (recip -> broadcast
    # -> O' normalize) contains DMA round-trips, so its emission is deferred
    # until after the NEXT head's units — otherwise its PE broadcast matmuls
    # sit in the PE stream at the head boundary and stall it for ~8us.
    onorms = []

    def finalize_head(j, osum):
        # (DMA moves the sums row to partition 0 first: DVE ops are
        # lane-locked and cannot read partition 64 while writing partition 0)
        recipT = ap_.tile([1, L], F32, name=f"recipT_{j}", tag="attn")
        nc.sync.dma_start(recipT[:], osum[DR : DR + 1, :])
        nc.vector.reciprocal_approx_fast(recipT[:], recipT[:])
        recipTr = ap_.tile([1, L], F32R, name=f"recipTr_{j}", tag="attn")
        nc.sync.dma_start(recipTr[:], recipT[:].bitcast(F32R))
        onorm = pp.tile([DR, L], F32R, name=f"onorm_{j}")
        onorms.append(onorm)
        for nh in range(2):
            bc = ps_tile(f"bc_{j}_{nh}")
            for n in range(2):
                nc.tensor.matmul(
                    bc[0:DR, n * 512 : (n + 1) * 512],
                    ones_col[:],
                    recipTr[:, nh * 1024 + n * 512 : nh * 1024 + (n + 1) * 512],
                    start=True,
                    stop=True,
                )
            nc.vector.tensor_mul(
                onorm[:, nh * 1024 : (nh + 1) * 1024],
                osum[0:DR, nh * 1024 : (nh + 1) * 1024],
                bc[0:DR, :],
            )

    pending = None
    for j in range(HPC):
        pb = 64 * (j % 2)
        mi = j // 2
        qh = qp[pb : pb + DR, mi, :]   # [64, 2048] Q_projT for head j
        kh = kp[pb : pb + DR, mi, :]   # [64, 2048] K_projT for head j

        osum = sp.tile([DR + 1, L], F32, name=f"osum_{j}", tag="osum", bufs=2)
        cps = None
        for u in range(CH):
            # --- transposed unit t=u: S^T chunk -> exp -> C matmuls ---
            t = u
            tb = t // 4
            if t % 4 == 0:
                cps = {nh: ps_tile(f"cps_{j}_{tb}_{nh}") for nh in range(2)}
            est = ep.tile([P, L], F32R, name=f"est_{j}_{t}", tag="est")
            for nh in range(2):
                st = ps_tile(f"st_{j}_{t}_{nh}")
                for n in range(2):
                    nc.tensor.matmul(
                        st[:, n * 512 : (n + 1) * 512],
                        kh[:, t * P : (t + 1) * P],
                        qh[:, nh * 1024 + n * 512 : nh * 1024 + (n + 1) * 512],
                        start=True,
                        stop=True,
                    )
                nc.scalar.activation(
                    est[:, nh * 1024 : (nh + 1) * 1024],
                    st[:],
                    AF.Exp,
                    scale=0.125,
                )
                # C: accumulate [V|1]^T @ expS^T
                for n in range(2):
                    nc.tensor.matmul(
                        cps[nh][0 : DR + 1, n * 512 : (n + 1) * 512],
                        vp[:, t, j, :],
                        est[:, nh * 1024 + n * 512 : nh * 1024 + (n + 1) * 512],
                        start=(t == tb * 4),
                        stop=(t == tb * 4 + 3),
                    )
            if t % 4 == 3:
                for nh in range(2):
                    dstp = osum[:, nh * 1024 : (nh + 1) * 1024]
                    if tb == 0:
                        nc.vector.tensor_copy(dstp, cps[nh][0 : DR + 1, :])
                    else:
                        nc.vector.tensor_add(dstp, dstp, cps[nh][0 : DR + 1, :])

            # --- natural unit c=u: S chunk -> exp+rowsum -> normalize -> out ---
            c = u
            attn = ap_.tile([P, L], F32, name=f"attn_{j}_{c}", tag="attn")
            sumh = sp.tile([P, 2], F32, name=f"sumh_{j}_{c}", tag="sumh")
            for nh in range(2):
                sps = ps_tile(f"sps_{j}_{c}_{nh}")
                for n in range(2):
                    nc.tensor.matmul(
                        sps[:, n * 512 : (n + 1) * 512],
                        qh[:, c * P : (c + 1) * P],
                        kh[:, nh * 1024 + n * 512 : nh * 1024 + (n + 1) * 512],
                        start=True,
                        stop=True,
                    )
                nc.scalar.activation(
                    attn[:, nh * 1024 : (nh + 1) * 1024],
                    sps[:],
                    AF.Exp,
                    scale=0.125,
                    accum_out=sumh[:, nh : nh + 1],
                )
            rc = sp.tile([P, 1], F32, name=f"rc_{j}_{c}", tag="rc")
            nc.vector.tensor_add(rc[:], sumh[:, 0:1], sumh[:, 1:2])
            nc.vector.reciprocal_approx_fast(rc[:], rc[:])
            nc.vector.tensor_scalar_mul(attn[:], attn[:], rc[:])
            nc.sync.dma_start(io["attn_o"][c * P : (c + 1) * P, j, :], attn[:])

            if u == 4 and pending is not None:
                finalize_head(*pending)
                pending = None
        pending = (j, osum)
    finalize_head(*pending)

    # ---- stage D: att_v partial = concat_h(O'_norm) @ Wc_slice ----
    for c in range(CH):
        dps = ps_tile(f"dps_{c}")
        for j in range(HPC):
            for n in range(2):
                nc.tensor.matmul(
                    dps[:, n * 512 : (n + 1) * 512],
                    onorms[j][:, c * P : (c + 1) * P],
                    wc_s[:, j, n * 512 : (n + 1) * 512],
                    start=(j == 0),
                    stop=(j == HPC - 1),
                )
        av = sp.tile([P, D], F32, name=f"av_{c}", tag="av")
        nc.vector.tensor_copy(av[:], dps[:])
        nc.sync.dma_start(io["attv_o"][c * P : (c + 1) * P, :], av[:])

    for pool in (psp, sp, ap_, ep, pp, xp, wp):
        pool.release()


_NC = None


def _build():
    global _NC
    if _NC is not None:
        return _NC
    nc = bacc.Bacc(trn_type="TRN2", target_bir_lowering=False, debug=False)
    io = {
        "qT": nc.dram_tensor("qT", [D, L], F32R, kind="ExternalInput").ap(),
        "kT": nc.dram_tensor("kT", [D, L], F32R, kind="ExternalInput").ap(),
        "vT": nc.dram_tensor("vT", [D, L], F32R, kind="ExternalInput").ap(),
        "wq": nc.dram_tensor("wq", [D, WC], F32R, kind="ExternalInput").ap(),
        "wk": nc.dram_tensor("wk", [D, WC], F32R, kind="ExternalInput").ap(),
        "wv": nc.dram_tensor("wv", [D, WC], F32R, kind="ExternalInput").ap(),
        "wc": nc.dram_tensor("wc", [WC, D], F32R, kind="ExternalInput").ap(),
        "attn_o": nc.dram_tensor(
            "attn_o", [L, HPC, L], F32, kind="ExternalOutput"
        ).ap(),
        "attv_o": nc.dram_tensor("attv_o", [L, D], F32, kind="ExternalOutput").ap(),
    }
    with tile.TileContext(nc) as tc:
        _emit(nc, tc, io)
    nc.compile()
    _NC = nc
    return nc


def kernel(q, k, v, Wq, Wk, Wv, Wc):
    q = np.asarray(q, np.float32)
    k = np.asarray(k, np.float32)
    v = np.asarray(v, np.float32)
    Wq = np.asarray(Wq, np.float32)
    Wk = np.asarray(Wk, np.float32)
    Wv = np.asarray(Wv, np.float32)
    Wc = np.asarray(Wc, np.float32)

    nc = _build()
    in_maps = []
    for c in range(NCORES):
        b, g = divmod(c, HPC)
        cols = slice(g * WC, (g + 1) * WC)
        in_maps.append(
            {
                "qT": np.ascontiguousarray(q[b].T),
                "kT": np.ascontiguousarray(k[b].T),
                "vT": np.ascontiguousarray(v[b].T),
                "wq": np.ascontiguousarray(Wq[:, cols]),
                "wk": np.ascontiguousarray(Wk[:, cols]),
                "wv": np.ascontiguousarray(Wv[:, cols]),
                "wc": np.ascontiguousarray(Wc[cols, :]),
            }
        )
    res = bass_utils.run_bass_kernel_spmd(nc, in_maps, core_ids=list(range(NCORES)))

    att_v = np.zeros((B, L, D), np.float32)
    att_groups = np.empty((B, L, R, L), np.float32)
    for c in range(NCORES):
        b, g = divmod(c, HPC)
        out = res.results[c]
        att_v[b] += out["attv_o"]
        att_groups[b, :, g * HPC : (g + 1) * HPC, :] = out["attn_o"]
    return att_v, att_groups


# revision 25
# speedup vs baseline: 1.2255x; 1.2255x over previous
"""Trainium2 Bass kernel for multi-head dot-product attention (R=16 heads,
B=2, L=2048, D=1024) returning (att_v, att_groups), sharded over 8 NeuronCores.

Sharding: core c handles batch b = c//4 and head group g = c%4 (4 heads).
Each core computes its heads' projections (column-sliced Wq/Wk/Wv), full
attention for those heads, the attention-map output slice, and a partial
att_v (row-sliced Wc); the host sums the 4 partials per batch.

On-device dataflow (per core), matmuls in float32r (reduced-precision fp32,
full PE rate at N=512):
  A:   Q_projT/K_projT/V_projT [256,2048] via PE, streaming qT/kT/vT k-chunks
       from DRAM; V_projT is PE-transposed into natural V [lk, 65]-per-head
       tiles (65th column = ones for the row-sum-via-matmul trick).
  B_T: S^T chunks [128lk, 2048lq] -> exp (ScalarE) -> C: PE accumulates
       [V|1]^T @ expS^T = [O'^T; rowsums], drained blockwise to SBUF.
  B:   natural S chunks [128lq, 2048lk] -> exp with accumulated row sums
       (ScalarE accum_out) -> per-partition normalize (VectorE) -> DMA out.
  D:   att_v partial = concat_h(O'_norm) @ Wc_slice from O'^T slices.

Softmax skips the max-subtraction: scores are ~N(0, 2.6^2) (max |s| ~ 12),
far inside fp32 exp range, and exp(s)/sum(exp(s)) is algebraically identical
to the max-shifted form.
"""

import numpy as np

import concourse.bass as bass
import concourse.mybir as mybir
import concourse.tile as tile
from concourse import bacc, bass_utils
from concourse.masks import make_identity

B, L, D, R = 2, 2048, 1024, 16
HPC = 4          # heads per core
DR = D // R      # 64
WC = HPC * DR    # 256 = projected width per core
NCORES = 8
CH = L // 128    # 16 chunks of 128 along lq/lk
P = 128
NKC = D // P     # 8 contraction chunks for projections
F32 = mybir.dt.float32
F32R = mybir.dt.float32r
BF16 = mybir.dt.bfloat16
AF = mybir.ActivationFunctionType


def _emit(nc, tc, io):
    wp = tc.alloc_tile_pool(name="wp", bufs=1)
    xp = tc.alloc_tile_pool(name="xp", bufs=2)
    pp = tc.alloc_tile_pool(name="pp", bufs=1)
    ep = tc.alloc_tile_pool(name="ep", bufs=3)
    ap_ = tc.alloc_tile_pool(name="ap", bufs=3)
    sp = tc.alloc_tile_pool(name="sp", bufs=2)
    psp = tc.alloc_tile_pool(name="psp", bufs=4, space="PSUM")

    def ps_tile(name):
        return psp.tile([P, 1024], F32, name=name, tag="s")

    # ---- weights to SBUF ----
    wq_s = wp.tile([P, NKC, WC], F32R)
    wk_s = wp.tile([P, NKC, WC], F32R)
    wv_s = wp.tile([P, NKC, WC], F32R)
    wc_s = wp.tile([DR, HPC, D], F32R)
    nc.sync.dma_start(wq_s[:], io["wq"].rearrange("(k p) m -> p k m", p=P))
    nc.sync.dma_start(wk_s[:], io["wk"].rearrange("(k p) m -> p k m", p=P))
    nc.sync.dma_start(wv_s[:], io["wv"].rearrange("(k p) m -> p k m", p=P))
    nc.sync.dma_start(wc_s[:], io["wc"].rearrange("(j p) m -> p j m", p=DR))
    ident = wp.tile([P, P], F32)
    make_identity(nc, ident[:])
    ones32 = wp.tile([P, CH * HPC], F32)
    nc.vector.memset(ones32[:], 1.0)
    ones_col = wp.tile([1, DR], F32R)
    nc.vector.tensor_copy(ones_col[:], ones32[0:1, 0:DR])

    # ---- stage A: projections (transposed layout) ----
    # qp/kp: [128, 2, 2048]; (p, m, l) = Proj_T[m*128 + p, l]; head j lives at
    # partitions 64*(j%2)..+64 of m-tile j//2.
    qp = pp.tile([P, 2, L], F32R)
    kp = pp.tile([P, 2, L], F32R)
    # vp: natural-layout V per (lk-chunk, head), augmented with ones column:
    # vp[p, t, j, 0:64] = V_proj[t*128+p, 64j:64j+64], vp[p, t, j, 64] = 1.0
    vp = pp.tile([P, CH, HPC, DR + 1], F32R)
    nc.vector.tensor_copy(
        vp[:, :, :, DR : DR + 1],
        ones32[:].rearrange("p (t j o) -> p t j o", t=CH, j=HPC),
    )

    for name, wsrc, xdram, dst in (
        ("q", wq_s, io["qT"], qp),
        ("k", wk_s, io["kT"], kp),
        ("v", wv_s, io["vT"], None),
    ):
        # 4 PSUM groups (m, nh), all alive across the k accumulation
        grp = {
            (m, nh): ps_tile(f"ps_{name}_{m}_{nh}")
            for m in range(2)
            for nh in range(2)
        }
        for ki in range(NKC):
            xt = xp.tile([P, L], F32R, name=f"xt_{name}_{ki}", tag="xt")
            nc.sync.dma_start(xt[:, 0:1024], xdram[ki * P : (ki + 1) * P, 0:1024])
            nc.sync.dma_start(xt[:, 1024:2048], xdram[ki * P : (ki + 1) * P, 1024:2048])
            for m in range(2):
                for nh in range(2):
                    for n in range(2):
                        nc.tensor.matmul(
                            grp[(m, nh)][:, n * 512 : (n + 1) * 512],
                            wsrc[:, ki, m * P : (m + 1) * P],
                            xt[:, nh * 1024 + n * 512 : nh * 1024 + (n + 1) * 512],
                            start=(ki == 0),
                            stop=(ki == NKC - 1),
                        )
        if dst is not None:
            for m in range(2):
                for nh in range(2):
                    nc.vector.tensor_copy(
                        dst[:, m, nh * 1024 : (nh + 1) * 1024], grp[(m, nh)][:]
                    )
        else:
            # V: drain to transposed staging (f32), PE-transpose into vp
            vpT = []
            for m in range(2):
                vt = xp.tile([P, L], F32, name=f"vpT_{m}", tag="xt")
                vpT.append(vt)
                for nh in range(2):
                    nc.vector.tensor_copy(
                        vt[:, nh * 1024 : (nh + 1) * 1024], grp[(m, nh)][:]
                    )
            for j in range(HPC):
                src = vpT[j // 2]
                pb = 64 * (j % 2)
                tps = ps_tile(f"tps_{j}")
                for t in range(CH):
                    # out = in.T via regular matmul against an identity block
                    # (rhs base_partition must match the input slice's base)
                    nc.tensor.matmul(
                        tps[:, t * DR : (t + 1) * DR],
                        src[pb : pb + DR, t * P : (t + 1) * P],
                        ident[pb : pb + DR, pb : pb + DR],
                        start=True,
                        stop=True,
                    )
                nc.vector.tensor_copy(
                    vp[:, :, j, 0:DR],
                    tps[:].rearrange("p (t d) -> p t d", d=DR),
                )

    # ---- per-head attention ----
    # The transposed (B_T -> exp -> C) chain and the natural (B -> exp ->
    # normalize -> DMA) chain are dependency-independent; interleave their
    # emission unit by unit so the PE always has runnable matmuls while the
    # other chain waits on ScalarE. The per-head finalize (recip -> broadcast
    # -> O' normalize) contains DMA round-trips, so its emission is deferred
    # until after the NEXT head's units — otherwise its PE broadcast matmuls
    # sit in the PE stream at the head boundary and stall it for ~8us.
    onorms = []

    def finalize_head(j, osum):
        # (DMA moves the sums row to partition 0 first: DVE ops are
        # lane-locked and cannot read partition 64 while writing partition 0)
        recipT = ap_.tile([1, L], F32, name=f"recipT_{j}", tag="attn")
        nc.sync.dma_start(recipT[:], osum[DR : DR + 1, :])
        nc.vector.reciprocal_approx_fast(recipT[:], recipT[:])
        recipTr = ap_.tile([1, L], F32R, name=f"recipTr_{j}", tag="attn")
        nc.sync.dma_start(recipTr[:], recipT[:].bitcast(F32R))
        onorm = pp.tile([DR, L], F32R, name=f"onorm_{j}")
        onorms.append(onorm)
        for nh in range(2):
            bc = ps_tile(f"bc_{j}_{nh}")
            for n in range(2):
                nc.tensor.matmul(
                    bc[0:DR, n * 512 : (n + 1) * 512],
                    ones_col[:],
                    recipTr[:, nh * 1024 + n * 512 : nh * 1024 + (n + 1) * 512],
                    start=True,
                    stop=True,
                )
            nc.vector.tensor_mul(
                onorm[:, nh * 1024 : (nh + 1) * 1024],
                osum[0:DR, nh * 1024 : (nh + 1) * 1024],
                bc[0:DR, :],
            )

    pending = None
    for j in range(HPC):
        pb = 64 * (j % 2)
        mi = j // 2
        qh = qp[pb : pb + DR, mi, :]   # [64, 2048] Q_projT for head j
        kh = kp[pb : pb + DR, mi, :]   # [64, 2048] K_projT for head j

        osum = sp.tile([DR + 1, L], F32, name=f"osum_{j}", tag="osum", bufs=2)
        cps = None
        ests = {}

        def emit_c(t):
            # C matmuls for unit t, emitted one unit late so est(t) is already
            # written by ScalarE and these run wait-free, back-to-back.
            nonlocal cps
            tb = t // 4
            if t % 4 == 0:
                cps = {nh: ps_tile(f"cps_{j}_{tb}_{nh}") for nh in range(2)}
            est = ests.pop(t)
            for nh in range(2):
                for n in range(2):
                    nc.tensor.matmul(
                        cps[nh][0 : DR + 1, n * 512 : (n + 1) * 512],
                        vp[:, t, j, :],
                        est[:, nh * 1024 + n * 512 : nh * 1024 + (n + 1) * 512],
                        start=(t == tb * 4),
                        stop=(t == tb * 4 + 3),
                    )
            if t % 4 == 3:
                for nh in range(2):
                    dstp = osum[:, nh * 1024 : (nh + 1) * 1024]
                    if tb == 0:
                        nc.vector.tensor_copy(dstp, cps[nh][0 : DR + 1, :])
                    else:
                        nc.vector.tensor_add(dstp, dstp, cps[nh][0 : DR + 1, :])

        for u in range(CH):
            # --- transposed unit t=u: S^T chunk -> exp ---
            t = u
            est = ep.tile([P, L], F32R, name=f"est_{j}_{t}", tag="est")
            ests[t] = est
            for nh in range(2):
                st = ps_tile(f"st_{j}_{t}_{nh}")
                for n in range(2):
                    nc.tensor.matmul(
                        st[:, n * 512 : (n + 1) * 512],
                        kh[:, t * P : (t + 1) * P],
                        qh[:, nh * 1024 + n * 512 : nh * 1024 + (n + 1) * 512],
                        start=True,
                        stop=True,
                    )
                nc.scalar.activation(
                    est[:, nh * 1024 : (nh + 1) * 1024],
                    st[:],
                    AF.Exp,
                    scale=0.125,
                )

            # --- natural unit c=u: S chunk -> exp+rowsum -> normalize -> out ---
            c = u
            attn = ap_.tile([P, L], F32, name=f"attn_{j}_{c}", tag="attn")
            sumh = sp.tile([P, 2], F32, name=f"sumh_{j}_{c}", tag="sumh")
            for nh in range(2):
                sps = ps_tile(f"sps_{j}_{c}_{nh}")
                for n in range(2):
                    nc.tensor.matmul(
                        sps[:, n * 512 : (n + 1) * 512],
                        qh[:, c * P : (c + 1) * P],
                        kh[:, nh * 1024 + n * 512 : nh * 1024 + (n + 1) * 512],
                        start=True,
                        stop=True,
                    )
                nc.scalar.activation(
                    attn[:, nh * 1024 : (nh + 1) * 1024],
                    sps[:],
                    AF.Exp,
                    scale=0.125,
                    accum_out=sumh[:, nh : nh + 1],
                )
            rc = sp.tile([P, 1], F32, name=f"rc_{j}_{c}", tag="rc")
            nc.vector.tensor_add(rc[:], sumh[:, 0:1], sumh[:, 1:2])
            nc.vector.reciprocal_approx_fast(rc[:], rc[:])
            nc.vector.tensor_scalar_mul(attn[:], attn[:], rc[:])
            nc.sync.dma_start(io["attn_o"][c * P : (c + 1) * P, j, :], attn[:])

            if u > 0:
                emit_c(u - 1)
            if u == 4 and pending is not None:
                finalize_head(*pending)
                pending = None
        emit_c(CH - 1)
        pending = (j, osum)
    finalize_head(*pending)

    # ---- stage D: att_v partial = concat_h(O'_norm) @ Wc_slice ----
    for c in range(CH):
        dps = ps_tile(f"dps_{c}")
        for j in range(HPC):
            for n in range(2):
                nc.tensor.matmul(
                    dps[:, n * 512 : (n + 1) * 512],
                    onorms[j][:, c * P : (c + 1) * P],
                    wc_s[:, j, n * 512 : (n + 1) * 512],
                    start=(j == 0),
                    stop=(j == HPC - 1),
                )
        av = ap_.tile([P, D], F32, name=f"av_{c}", tag="attn")
        nc.vector.tensor_copy(av[:], dps[:])
        nc.sync.dma_start(io["attv_o"][c * P : (c + 1) * P, :], av[:])

    for pool in (psp, sp, ap_, ep, pp, xp, wp):
        pool.release()


_NC = None


def _build():
    global _NC
    if _NC is not None:
        return _NC
    nc = bacc.Bacc(trn_type="TRN2", target_bir_lowering=False, debug=False)
    io = {
        "qT": nc.dram_tensor("qT", [D, L], F32R, kind="ExternalInput").ap(),
        "kT": nc.dram_tensor("kT", [D, L], F32R, kind="ExternalInput").ap(),
        "vT": nc.dram_tensor("vT", [D, L], F32R, kind="ExternalInput").ap(),
        "wq": nc.dram_tensor("wq", [D, WC], F32R, kind="ExternalInput").ap(),
        "wk": nc.dram_tensor("wk", [D, WC], F32R, kind="ExternalInput").ap(),
        "wv": nc.dram_tensor("wv", [D, WC], F32R, kind="ExternalInput").ap(),
        "wc": nc.dram_tensor("wc", [WC, D], F32R, kind="ExternalInput").ap(),
        "attn_o": nc.dram_tensor(
            "attn_o", [L, HPC, L], F32, kind="ExternalOutput"
        ).ap(),
        "attv_o": nc.dram_tensor("attv_o", [L, D], F32, kind="ExternalOutput").ap(),
    }
    with tile.TileContext(nc) as tc:
        _emit(nc, tc, io)
    nc.compile()
    _NC = nc
    return nc


def kernel(q, k, v, Wq, Wk, Wv, Wc):
    q = np.asarray(q, np.float32)
    k = np.asarray(k, np.float32)
    v = np.asarray(v, np.float32)
    Wq = np.asarray(Wq, np.float32)
    Wk = np.asarray(Wk, np.float32)
    Wv = np.asarray(Wv, np.float32)
    Wc = np.asarray(Wc, np.float32)

    nc = _build()
    in_maps = []
    for c in range(NCORES):
        b, g = divmod(c, HPC)
        cols = slice(g * WC, (g + 1) * WC)
        in_maps.append(
            {
                "qT": np.ascontiguousarray(q[b].T),
                "kT": np.ascontiguousarray(k[b].T),
                "vT": np.ascontiguousarray(v[b].T),
                "wq": np.ascontiguousarray(Wq[:, cols]),
                "wk": np.ascontiguousarray(Wk[:, cols]),
                "wv": np.ascontiguousarray(Wv[:, cols]),
                "wc": np.ascontiguousarray(Wc[cols, :]),
            }
        )
    res = bass_utils.run_bass_kernel_spmd(nc, in_maps, core_ids=list(range(NCORES)))

    att_v = np.zeros((B, L, D), np.float32)
    att_groups = np.empty((B, L, R, L), np.float32)
    for c in range(NCORES):
        b, g = divmod(c, HPC)
        out = res.results[c]
        att_v[b] += out["attv_o"]
        att_groups[b, :, g * HPC : (g + 1) * HPC, :] = out["attn_o"]
    return att_v, att_groups


# revision 26
# speedup vs baseline: 1.2747x; 1.0401x over previous
"""Trainium2 Bass kernel for multi-head dot-product attention (R=16 heads,
B=2, L=2048, D=1024) returning (att_v, att_groups), sharded over 8 NeuronCores.

Sharding: core c handles batch b = c//4 and head group g = c%4 (4 heads).
Each core computes its heads' projections (column-sliced Wq/Wk/Wv), full
attention for those heads, the attention-map output slice, and a partial
att_v (row-sliced Wc); the host sums the 4 partials per batch.

On-device dataflow (per core), matmuls in float32r (reduced-precision fp32,
full PE rate at N=512):
  A:   Q_projT/K_projT/V_projT [256,2048] via PE, streaming qT/kT/vT k-chunks
       from DRAM; V_projT is PE-transposed into natural V [lk, 65]-per-head
       tiles (65th column = ones for the row-sum-via-matmul trick).
  B_T: S^T chunks [128lk, 2048lq] -> exp (ScalarE) -> C: PE accumulates
       [V|1]^T @ expS^T = [O'^T; rowsums], drained blockwise to SBUF.
  B:   natural S chunks [128lq, 2048lk] -> exp with accumulated row sums
       (ScalarE accum_out) -> per-partition normalize (VectorE) -> DMA out.
  D:   att_v partial = concat_h(O'_norm) @ Wc_slice from O'^T slices.

Softmax skips the max-subtraction: scores are ~N(0, 2.6^2) (max |s| ~ 12),
far inside fp32 exp range, and exp(s)/sum(exp(s)) is algebraically identical
to the max-shifted form.
"""

import ml_dtypes
import numpy as np

import concourse.bass as bass
import concourse.mybir as mybir
import concourse.tile as tile
from concourse import bacc, bass_utils
from concourse.masks import make_identity

B, L, D, R = 2, 2048, 1024, 16
HPC = 4          # heads per core
DR = D // R      # 64
WC = HPC * DR    # 256 = projected width per core
NCORES = 8
CH = L // 128    # 16 chunks of 128 along lq/lk
P = 128
NKC = D // P     # 8 contraction chunks for projections
F32 = mybir.dt.float32
F32R = mybir.dt.float32r
BF16 = mybir.dt.bfloat16
AF = mybir.ActivationFunctionType


def _emit(nc, tc, io):
    wp = tc.alloc_tile_pool(name="wp", bufs=1)
    xp = tc.alloc_tile_pool(name="xp", bufs=2)
    pp = tc.alloc_tile_pool(name="pp", bufs=1)
    ep = tc.alloc_tile_pool(name="ep", bufs=3)
    ap_ = tc.alloc_tile_pool(name="ap", bufs=3)
    sp = tc.alloc_tile_pool(name="sp", bufs=2)
    psp = tc.alloc_tile_pool(name="psp", bufs=4, space="PSUM")

    def ps_tile(name):
        return psp.tile([P, 1024], F32, name=name, tag="s")

    # ---- weights to SBUF ----
    wq_s = wp.tile([P, NKC, WC], F32R)
    wk_s = wp.tile([P, NKC, WC], F32R)
    wv_s = wp.tile([P, NKC, WC], F32R)
    wc_s = wp.tile([DR, HPC, D], BF16)
    nc.sync.dma_start(wq_s[:], io["wq"].rearrange("(k p) m -> p k m", p=P))
    nc.sync.dma_start(wk_s[:], io["wk"].rearrange("(k p) m -> p k m", p=P))
    nc.sync.dma_start(wv_s[:], io["wv"].rearrange("(k p) m -> p k m", p=P))
    nc.sync.dma_start(wc_s[:], io["wc"].rearrange("(j p) m -> p j m", p=DR))
    ident = wp.tile([P, P], F32)
    make_identity(nc, ident[:])
    ones32 = wp.tile([P, CH * HPC], F32)
    nc.vector.memset(ones32[:], 1.0)
    ones_col = wp.tile([1, DR], F32R)
    nc.vector.tensor_copy(ones_col[:], ones32[0:1, 0:DR])

    # ---- stage A: projections (transposed layout) ----
    # qp/kp: [128, 2, 2048]; (p, m, l) = Proj_T[m*128 + p, l]; head j lives at
    # partitions 64*(j%2)..+64 of m-tile j//2.
    qp = pp.tile([P, 2, L], BF16)
    kp = pp.tile([P, 2, L], BF16)
    # vp: natural-layout V per (lk-chunk, head), augmented with ones column:
    # vp[p, t, j, 0:64] = V_proj[t*128+p, 64j:64j+64], vp[p, t, j, 64] = 1.0
    vp = pp.tile([P, CH, HPC, DR + 1], BF16)
    nc.vector.tensor_copy(
        vp[:, :, :, DR : DR + 1],
        ones32[:].rearrange("p (t j o) -> p t j o", t=CH, j=HPC),
    )

    for name, wsrc, xdram, dst in (
        ("q", wq_s, io["qT"], qp),
        ("k", wk_s, io["kT"], kp),
        ("v", wv_s, io["vT"], None),
    ):
        # 4 PSUM groups (m, nh), all alive across the k accumulation
        grp = {
            (m, nh): ps_tile(f"ps_{name}_{m}_{nh}")
            for m in range(2)
            for nh in range(2)
        }
        for ki in range(NKC):
            xt = xp.tile([P, L], F32R, name=f"xt_{name}_{ki}", tag="xt")
            nc.sync.dma_start(xt[:, 0:1024], xdram[ki * P : (ki + 1) * P, 0:1024])
            nc.sync.dma_start(xt[:, 1024:2048], xdram[ki * P : (ki + 1) * P, 1024:2048])
            for m in range(2):
                for nh in range(2):
                    for n in range(2):
                        nc.tensor.matmul(
                            grp[(m, nh)][:, n * 512 : (n + 1) * 512],
                            wsrc[:, ki, m * P : (m + 1) * P],
                            xt[:, nh * 1024 + n * 512 : nh * 1024 + (n + 1) * 512],
                            start=(ki == 0),
                            stop=(ki == NKC - 1),
                        )
        if dst is not None:
            for m in range(2):
                for nh in range(2):
                    nc.vector.tensor_copy(
                        dst[:, m, nh * 1024 : (nh + 1) * 1024], grp[(m, nh)][:]
                    )
        else:
            # V: drain to transposed staging (f32), PE-transpose into vp
            vpT = []
            for m in range(2):
                vt = xp.tile([P, L], F32, name=f"vpT_{m}", tag="xt")
                vpT.append(vt)
                for nh in range(2):
                    nc.vector.tensor_copy(
                        vt[:, nh * 1024 : (nh + 1) * 1024], grp[(m, nh)][:]
                    )
            for j in range(HPC):
                src = vpT[j // 2]
                pb = 64 * (j % 2)
                tps = ps_tile(f"tps_{j}")
                for t in range(CH):
                    # out = in.T via regular matmul against an identity block
                    # (rhs base_partition must match the input slice's base)
                    nc.tensor.matmul(
                        tps[:, t * DR : (t + 1) * DR],
                        src[pb : pb + DR, t * P : (t + 1) * P],
                        ident[pb : pb + DR, pb : pb + DR],
                        start=True,
                        stop=True,
                    )
                nc.vector.tensor_copy(
                    vp[:, :, j, 0:DR],
                    tps[:].rearrange("p (t d) -> p t d", d=DR),
                )

    # ---- per-head attention ----
    # The transposed (B_T -> exp -> C) chain and the natural (B -> exp ->
    # normalize -> DMA) chain are dependency-independent; interleave their
    # emission unit by unit so the PE always has runnable matmuls while the
    # other chain waits on ScalarE. The per-head finalize (recip -> broadcast
    # -> O' normalize) contains DMA round-trips, so its emission is deferred
    # until after the NEXT head's units — otherwise its PE broadcast matmuls
    # sit in the PE stream at the head boundary and stall it for ~8us.
    onorms = []

    def finalize_head(j, osum):
        # (DMA moves the sums row to partition 0 first: DVE ops are
        # lane-locked and cannot read partition 64 while writing partition 0)
        recipT = ap_.tile([1, L], F32, name=f"recipT_{j}", tag="attn")
        nc.sync.dma_start(recipT[:], osum[DR : DR + 1, :])
        nc.vector.reciprocal_approx_fast(recipT[:], recipT[:])
        recipTr = ap_.tile([1, L], F32R, name=f"recipTr_{j}", tag="attn")
        nc.sync.dma_start(recipTr[:], recipT[:].bitcast(F32R))
        onorm = pp.tile([DR, L], BF16, name=f"onorm_{j}")
        onorms.append(onorm)
        for nh in range(2):
            bc = ps_tile(f"bc_{j}_{nh}")
            for n in range(2):
                nc.tensor.matmul(
                    bc[0:DR, n * 512 : (n + 1) * 512],
                    ones_col[:],
                    recipTr[:, nh * 1024 + n * 512 : nh * 1024 + (n + 1) * 512],
                    start=True,
                    stop=True,
                )
            nc.vector.tensor_mul(
                onorm[:, nh * 1024 : (nh + 1) * 1024],
                osum[0:DR, nh * 1024 : (nh + 1) * 1024],
                bc[0:DR, :],
            )

    pending = None
    for j in range(HPC):
        pb = 64 * (j % 2)
        mi = j // 2
        qh = qp[pb : pb + DR, mi, :]   # [64, 2048] Q_projT for head j
        kh = kp[pb : pb + DR, mi, :]   # [64, 2048] K_projT for head j

        osum = sp.tile([DR + 1, L], F32, name=f"osum_{j}", tag="osum", bufs=2)
        cps = None
        ests = {}

        def emit_c(t):
            # C matmuls for unit t, emitted one unit late so est(t) is already
            # written by ScalarE and these run wait-free, back-to-back.
            nonlocal cps
            tb = t // 4
            if t % 4 == 0:
                cps = {nh: ps_tile(f"cps_{j}_{tb}_{nh}") for nh in range(2)}
            est = ests.pop(t)
            for nh in range(2):
                for n in range(2):
                    nc.tensor.matmul(
                        cps[nh][0 : DR + 1, n * 512 : (n + 1) * 512],
                        vp[:, t, j, :],
                        est[:, nh * 1024 + n * 512 : nh * 1024 + (n + 1) * 512],
                        start=(t == tb * 4),
                        stop=(t == tb * 4 + 3),
                    )
            if t % 4 == 3:
                for nh in range(2):
                    dstp = osum[:, nh * 1024 : (nh + 1) * 1024]
                    if tb == 0:
                        nc.vector.tensor_copy(dstp, cps[nh][0 : DR + 1, :])
                    else:
                        nc.vector.tensor_add(dstp, dstp, cps[nh][0 : DR + 1, :])

        for u in range(CH):
            # --- transposed unit t=u: S^T chunk -> exp ---
            t = u
            est = ep.tile([P, L], BF16, name=f"est_{j}_{t}", tag="est")
            ests[t] = est
            for nh in range(2):
                st = ps_tile(f"st_{j}_{t}_{nh}")
                for n in range(2):
                    nc.tensor.matmul(
                        st[:, n * 512 : (n + 1) * 512],
                        kh[:, t * P : (t + 1) * P],
                        qh[:, nh * 1024 + n * 512 : nh * 1024 + (n + 1) * 512],
                        start=True,
                        stop=True,
                    )
                nc.scalar.activation(
                    est[:, nh * 1024 : (nh + 1) * 1024],
                    st[:],
                    AF.Exp,
                    scale=0.125,
                )

            # --- natural unit c=u: S chunk -> exp+rowsum -> normalize -> out ---
            c = u
            attn = ap_.tile([P, L], F32, name=f"attn_{j}_{c}", tag="attn")
            sumh = sp.tile([P, 2], F32, name=f"sumh_{j}_{c}", tag="sumh")
            for nh in range(2):
                sps = ps_tile(f"sps_{j}_{c}_{nh}")
                for n in range(2):
                    nc.tensor.matmul(
                        sps[:, n * 512 : (n + 1) * 512],
                        qh[:, c * P : (c + 1) * P],
                        kh[:, nh * 1024 + n * 512 : nh * 1024 + (n + 1) * 512],
                        start=True,
                        stop=True,
                    )
                nc.scalar.activation(
                    attn[:, nh * 1024 : (nh + 1) * 1024],
                    sps[:],
                    AF.Exp,
                    scale=0.125,
                    accum_out=sumh[:, nh : nh + 1],
                )
            rc = sp.tile([P, 1], F32, name=f"rc_{j}_{c}", tag="rc")
            nc.vector.tensor_add(rc[:], sumh[:, 0:1], sumh[:, 1:2])
            nc.vector.reciprocal_approx_fast(rc[:], rc[:])
            nc.vector.tensor_scalar_mul(attn[:], attn[:], rc[:])
            nc.sync.dma_start(io["attn_o"][c * P : (c + 1) * P, j, :], attn[:])

            if u > 0:
                emit_c(u - 1)
            if u == 4 and pending is not None:
                finalize_head(*pending)
                pending = None
        emit_c(CH - 1)
        pending = (j, osum)
    finalize_head(*pending)

    # ---- stage D: att_v partial = concat_h(O'_norm) @ Wc_slice ----
    for c in range(CH):
        dps = ps_tile(f"dps_{c}")
        for j in range(HPC):
            for n in range(2):
                nc.tensor.matmul(
                    dps[:, n * 512 : (n + 1) * 512],
                    onorms[j][:, c * P : (c + 1) * P],
                    wc_s[:, j, n * 512 : (n + 1) * 512],
                    start=(j == 0),
                    stop=(j == HPC - 1),
                )
        av = ap_.tile([P, D], F32, name=f"av_{c}", tag="attn")
        nc.vector.tensor_copy(av[:], dps[:])
        nc.sync.dma_start(io["attv_o"][c * P : (c + 1) * P, :], av[:])

    for pool in (psp, sp, ap_, ep, pp, xp, wp):
        pool.release()


_NC = None


def _build():
    global _NC
    if _NC is not None:
        return _NC
    nc = bacc.Bacc(trn_type="TRN2", target_bir_lowering=False, debug=False)
    io = {
        "qT": nc.dram_tensor("qT", [D, L], F32R, kind="ExternalInput").ap(),
        "kT": nc.dram_tensor("kT", [D, L], F32R, kind="ExternalInput").ap(),
        "vT": nc.dram_tensor("vT", [D, L], F32R, kind="ExternalInput").ap(),
        "wq": nc.dram_tensor("wq", [D, WC], F32R, kind="ExternalInput").ap(),
        "wk": nc.dram_tensor("wk", [D, WC], F32R, kind="ExternalInput").ap(),
        "wv": nc.dram_tensor("wv", [D, WC], F32R, kind="ExternalInput").ap(),
        "wc": nc.dram_tensor("wc", [WC, D], BF16, kind="ExternalInput").ap(),
        "attn_o": nc.dram_tensor(
            "attn_o", [L, HPC, L], F32, kind="ExternalOutput"
        ).ap(),
        "attv_o": nc.dram_tensor("attv_o", [L, D], F32, kind="ExternalOutput").ap(),
    }
    with tile.TileContext(nc) as tc:
        _emit(nc, tc, io)
    nc.compile()
    _NC = nc
    return nc


def kernel(q, k, v, Wq, Wk, Wv, Wc):
    q = np.asarray(q, np.float32)
    k = np.asarray(k, np.float32)
    v = np.asarray(v, np.float32)
    Wq = np.asarray(Wq, np.float32)
    Wk = np.asarray(Wk, np.float32)
    Wv = np.asarray(Wv, np.float32)
    Wc = np.asarray(Wc, np.float32)

    nc = _build()
    in_maps = []
    for c in range(NCORES):
        b, g = divmod(c, HPC)
        cols = slice(g * WC, (g + 1) * WC)
        in_maps.append(
            {
                "qT": np.ascontiguousarray(q[b].T),
                "kT": np.ascontiguousarray(k[b].T),
                "vT": np.ascontiguousarray(v[b].T),
                "wq": np.ascontiguousarray(Wq[:, cols]),
                "wk": np.ascontiguousarray(Wk[:, cols]),
                "wv": np.ascontiguousarray(Wv[:, cols]),
                "wc": np.ascontiguousarray(Wc[cols, :]).astype(ml_dtypes.bfloat16),
            }
        )
    res = bass_utils.run_bass_kernel_spmd(nc, in_maps, core_ids=list(range(NCORES)))

    att_v = np.zeros((B, L, D), np.float32)
    att_groups = np.empty((B, L, R, L), np.float32)
    for c in range(NCORES):
        b, g = divmod(c, HPC)
        out = res.results[c]
        att_v[b] += out["attv_o"]
        att_groups[b, :, g * HPC : (g + 1) * HPC, :] = out["attn_o"]
    return att_v, att_groups
